# revision 21
# baseline (speedup 1.0000x reference)
"""Trainium2 Bass kernel for a track-wise (ragged-sequence) attention layer.

Math (per track t of length L, per head h):
    qkv = values @ w_qkv.T + b_qkv                      # [N, 3*256]
    S   = q k^T / sqrt(Dh);  P = softmax(S);  ctx = P v
    out = ctx @ w_lin.T + b_lin

Device strategy (data parallel over tracks, 8 cores, no cross-core comm):
  - scores are computed directly transposed per head: ST = K @ Q^T, so
    exp(ST) is exactly the lhs^T the ctx matmul needs -- no PE transposes.
  - no max-subtraction in softmax (scores are ~N(0,1); exp is safe in fp32);
    the denominator is recovered with a ones-matmul over exp(ST) and applied
    as one divide per head-group on the ctx accumulator.
  - 1/sqrt(Dh) is folded into w_q/b_q on the host; b_v is folded into the
    final bias (softmax rows sum to 1): b_final = w_lin @ b_v + b_lin.
  - all matmuls run as float32r (1 cycle/row on TRN2 for moving dim >= 256).
"""

import os
import sys

import numpy as np

for _p in ("/opt/trn_rl_repo", "/root/.axon_site/_ro/trn_rl_repo"):
    if os.path.isdir(_p) and _p not in sys.path:
        sys.path.append(_p)

import concourse.bass as bass
import concourse.tile as tile
from concourse import mybir
from concourse.vector_clock import ScopedClock
from concourse.bass_utils import run_bass_kernel_spmd

F32 = mybir.dt.float32
F32R = mybir.dt.float32r
EXP = mybir.ActivationFunctionType.Exp
LOG = mybir.ActivationFunctionType.Ln
MUL = mybir.AluOpType.mult
ADD = mybir.AluOpType.add
DIV = mybir.AluOpType.divide

N_CORES = 8
N, DIN, DOUT, H, T, L = 65536, 256, 256, 8, 256, 256
DH = DOUT // H          # 32
PC = N // N_CORES       # 8192 points per core
TPC = T // N_CORES      # 32 tracks per core
TPG = 4                 # tracks per group
NG = TPC // TPG         # 8 groups
GP = TPG * L            # 1024 points per group
MC_G = GP // 128        # 8 128-point chunks per group


class _TileContext(tile.TileContext):
    """TileContext whose final drain carries at most one semaphore wait per
    instruction (the walrus build in this container rejects multi-wait
    TPB_CTRL instructions)."""

    def _drain_and_barrier(self, tick_clock, wait_clock):
        super()._drain_and_barrier(tick_clock, wait_clock)
        self._split_multi_waits()

    def _split_multi_waits(self):
        nc = self.nc
        for f in nc.m.functions:
            for bb in f.blocks:
                changed = False
                new_insts = []
                for inst in bb.instructions:
                    si = inst.sync_info
                    if si is not None and len(si.on_wait) > 1:
                        waits = list(si.on_wait)
                        for w in waits[:-1]:
                            nop = mybir.InstNoOp(
                                name=f"I-{nc.next_id()}", ins=[], outs=[]
                            )
                            nop.engine = inst.engine
                            nop.sync_info = mybir.SyncInfo(
                                on_wait=[w], on_update=[]
                            )
                            new_insts.append(nop)
                        inst.sync_info = mybir.SyncInfo(
                            on_wait=[waits[-1]], on_update=list(si.on_update)
                        )
                        changed = True
                    new_insts.append(inst)
                if changed:
                    bb.instructions = new_insts


def _r(ap):
    return ap.bitcast(F32R)


def _ensure_ntff_hook():
    """The agent image's ``antenv`` lacks ``axon_hooks``; provide it so
    ``run_bass_kernel_spmd(trace=True)`` can register the axon NTFF hook."""
    try:
        import antenv.axon_hooks  # noqa: F401
        return
    except ImportError:
        pass
    import types

    import antenv

    mod = types.ModuleType("antenv.axon_hooks")
    _hook = [None]
    mod.set_axon_ntff_profile_hook = lambda h: _hook.__setitem__(0, h)
    mod.get_axon_ntff_profile_hook = lambda: _hook[0]
    sys.modules["antenv.axon_hooks"] = mod
    antenv.axon_hooks = mod
    try:
        from trn_agent_boot.trn_boot import _ntff_profile_via_ctypes

        mod.set_axon_ntff_profile_hook(
            _ntff_profile_via_ctypes("/opt/axon/libaxon_pjrt.so")
        )
    except Exception as e:  # pragma: no cover - tracing is best-effort
        print(f"ntff hook setup failed: {e}", file=sys.stderr)


def _build_program(masked: bool):
    nc = bass.Bass("TRN2", target_bir_lowering=False, debug=False,
                   num_devices=N_CORES)

    xT = nc.dram_tensor("xT", [2, 128, PC], F32R, kind="ExternalInput").ap()
    wqk = nc.dram_tensor("wqk", [2, 128, 512], F32R, kind="ExternalInput").ap()
    wv = nc.dram_tensor("wv", [2, 128, 256], F32R, kind="ExternalInput").ap()
    wl = nc.dram_tensor("wl", [2, 128, 256], F32R, kind="ExternalInput").ap()
    bqk = nc.dram_tensor("bqk", [128, 4], F32, kind="ExternalInput").ap()
    bfin = nc.dram_tensor("bfin", [256], F32, kind="ExternalInput").ap()
    if masked:
        mkf = nc.dram_tensor("mkf", [PC], F32, kind="ExternalInput").ap()
        mkp = nc.dram_tensor("mkp", [128, PC // 128], F32,
                             kind="ExternalInput").ap()
    out = nc.dram_tensor("out", [PC, DOUT], F32, kind="ExternalOutput").ap()

    with _TileContext(nc) as tc:
        with (
            tc.tile_pool(name="consts", bufs=1) as consts,
            tc.tile_pool(name="xg", bufs=2) as xg_pool,
            tc.tile_pool(name="qk", bufs=2) as qk_pool,
            tc.tile_pool(name="vsb", bufs=2) as v_pool,
            tc.tile_pool(name="et", bufs=3) as et_pool,
            tc.tile_pool(name="ctxT", bufs=2) as ctx_pool,
            tc.tile_pool(name="osb", bufs=2) as o_pool,
            tc.tile_pool(name="ps_st", bufs=2, space="PSUM") as ps_st,
            tc.tile_pool(name="ps_c4", bufs=2, space="PSUM") as ps_c4,
        ):
            # ---- constants -------------------------------------------------
            wqk_sb = consts.tile([128, 2, 512], F32R)
            nc.gpsimd.dma_start(out=wqk_sb, in_=wqk.rearrange("k p r -> p k r"))
            wv_sb = consts.tile([128, 2, 256], F32R)
            nc.gpsimd.dma_start(out=wv_sb, in_=wv.rearrange("k p r -> p k r"))
            wl_sb = consts.tile([128, 2, 256], F32R)
            nc.gpsimd.dma_start(out=wl_sb, in_=wl.rearrange("k p r -> p k r"))
            bqk_sb = consts.tile([128, 4], F32)
            nc.gpsimd.dma_start(out=bqk_sb, in_=bqk)
            bfin_sb = consts.tile([128, 256], F32)
            nc.gpsimd.dma_start(out=bfin_sb, in_=bfin.partition_broadcast(128))
            if masked:
                mkp_sb = consts.tile([128, PC // 128], F32)
                nc.gpsimd.dma_start(out=mkp_sb, in_=mkp)

            def emit_load(g):
                gsl = slice(g * GP, (g + 1) * GP)
                xg = xg_pool.tile([128, 2, GP], F32R, tag="xg", name=f"xg{g}")
                nc.gpsimd.dma_start(
                    out=xg, in_=xT[:, :, gsl].rearrange("k p n -> p k n")
                )
                mk_sb = None
                if masked:
                    mk_sb = et_pool.tile([128, GP], F32, tag="mk",
                                         name=f"mk{g}")
                    nc.gpsimd.dma_start(
                        out=mk_sb, in_=mkf[gsl].partition_broadcast(128)
                    )
                return xg, mk_sb

            def emit_a(g, xg, mk_sb):
                # A1: Q^T/K^T rows [512, GP]
                qk_sb = qk_pool.tile([128, 4, GP], F32R, tag="qk", name=f"qk{g}")
                for rr in range(4):
                    for n2 in range(GP // 512):
                        ps = ps_c4.tile([128, 512], F32, tag="c4",
                                        name=f"psa{g}_{rr}_{n2}")
                        for k in range(2):
                            nc.tensor.matmul(
                                ps,
                                wqk_sb[:, k, rr * 128:(rr + 1) * 128],
                                xg[:, k, n2 * 512:(n2 + 1) * 512],
                                start=(k == 0), stop=(k == 1),
                            )
                        nc.vector.tensor_scalar_add(
                            qk_sb[:, rr, n2 * 512:(n2 + 1) * 512],
                            ps, bqk_sb[:, rr:rr + 1],
                        )
                if masked:
                    for rr in (2, 3):  # zero K^T columns of padded slots
                        nc.vector.tensor_tensor(
                            qk_sb[:, rr, :], qk_sb[:, rr, :], mk_sb, MUL
                        )
                # A2: V -> v_aug [p, mc, head, [V(32) | ones(32)]]
                v_aug = v_pool.tile([128, MC_G, H, 64], F32R, tag="va", name=f"va{g}")
                nc.gpsimd.memset(v_aug[:, :, :, 32:64].bitcast(F32), 1.0)
                for mc in range(MC_G):
                    ps = ps_c4.tile([128, 512], F32, tag="c4",
                                    name=f"psv{g}_{mc}")
                    for k in range(2):
                        nc.tensor.matmul(
                            ps[:, 0:256],
                            xg[:, k, mc * 128:(mc + 1) * 128],
                            wv_sb[:, k, :],
                            start=(k == 0), stop=(k == 1),
                        )
                    if masked:
                        nc.vector.tensor_scalar_mul(
                            v_aug[:, mc, :, 0:32], ps[:, 0:256].rearrange(
                                "p (h d) -> p h d", h=H),
                            mkp_sb[:, g * MC_G + mc:g * MC_G + mc + 1],
                        )
                    else:
                        nc.vector.tensor_copy(
                            v_aug[:, mc, :, 0:32],
                            ps[:, 0:256].rearrange("p (h d) -> p h d", h=H),
                        )
                return qk_sb, v_aug

            def emit_st_exp(g, qk_sb, t, hg):
                # ST = K@Q^T then exp, two 2-head sub-batches
                tsl = slice(t * 256, (t + 1) * 256)
                et4 = et_pool.tile([128, 4, 512], F32R, tag="et",
                                   name=f"et{g}_{t}_{hg}")
                st2s = [ps_st.tile([128, 2, 512], F32, tag="st",
                                   name=f"st{g}_{t}_{hg}_{sb}")
                        for sb in range(2)]
                for j in range(2):
                    for hh in range(4):
                        po = hh * 32
                        nc.tensor.matmul(
                            st2s[hh // 2][:, hh % 2, j * 256:(j + 1) * 256],
                            qk_sb[po:po + 32, 2 + hg,
                                  t * 256 + j * 128:t * 256 + (j + 1) * 128],
                            qk_sb[po:po + 32, hg, tsl],
                            start=True, stop=True,
                            tile_position=(po, 0),
                        )
                for sb in range(2):
                    nc.scalar.activation(
                        et4[:, sb * 2:(sb + 1) * 2, :], st2s[sb], EXP
                    )
                return et4

            def emit_ctx(g, v_aug, t, hg, et4):
                acc4 = ps_c4.tile([64, 4, 256], F32, tag="c4",
                                  name=f"acc{g}_{t}_{hg}")
                for hh in range(4):
                    h = hg * 4 + hh
                    for j in range(2):
                        nc.tensor.matmul(
                            acc4[:, hh, :],
                            v_aug[:, t * 2 + j, h, :],
                            et4[:, hh, j * 256:(j + 1) * 256],
                            start=(j == 0), stop=(j == 1),
                        )
                return acc4

            def emit_norm(g, ctxT_sb, t, hg, acc4):
                tsl = slice(t * 256, (t + 1) * 256)
                ln4 = et_pool.tile([32, 4, 256], F32, tag="ln",
                                   name=f"ln{g}_{t}_{hg}")
                nc.scalar.activation(ln4, acc4[32:64, :, :], LOG)
                rcp4 = et_pool.tile([32, 4, 256], F32, tag="rcp",
                                    name=f"rcp{g}_{t}_{hg}")
                nc.scalar.activation(rcp4, ln4, EXP, scale=-1.0)
                for hh in range(4):
                    nc.vector.tensor_tensor(
                        ctxT_sb[hh * 32:(hh + 1) * 32, hg, tsl],
                        acc4[0:32, hh, :], rcp4[:, hh, :], MUL
                    )

            def emit_c(g, ctxT_sb):
                gsl = slice(g * GP, (g + 1) * GP)
                o_sb = o_pool.tile([128, MC_G, 256], F32, tag="o", name=f"o{g}")
                for mc in range(MC_G):
                    ps = ps_c4.tile([128, 512], F32, tag="c4",
                                    name=f"psc{g}_{mc}")
                    for kc in range(2):
                        nc.tensor.matmul(
                            ps[:, 0:256],
                            ctxT_sb[:, kc, mc * 128:(mc + 1) * 128],
                            wl_sb[:, kc, :],
                            start=(kc == 0), stop=(kc == 1),
                        )
                    nc.vector.tensor_tensor(
                        o_sb[:, mc, :], ps[:, 0:256], bfin_sb, ADD
                    )
                nc.gpsimd.dma_start(
                    out=out[gsl, :].rearrange("(m p) n -> p m n", p=128),
                    in_=o_sb,
                )

            # software-pipelined schedule: the PE stream never waits on the
            # same iteration's exp -- ST MMs of iter i+1 are emitted before
            # the ctx MMs of iter i; the next group's A-phase is hoisted
            # before the current group's C-phase.
            xg0 = emit_load(0)
            qk, va = emit_a(0, *xg0)
            ab = {0: (qk, va)}
            iters = [(t, hg) for t in range(TPG) for hg in (0, 1)]
            for g in range(NG):
                qk_sb, v_aug = ab.pop(g)
                ctxT_sb = ctx_pool.tile([128, 2, GP], F32R, tag="ctxT", name=f"ctxT{g}")
                if g + 1 < NG:
                    xgn = emit_load(g + 1)
                pend = None
                for i, (t, hg) in enumerate(iters):
                    et4 = emit_st_exp(g, qk_sb, t, hg)
                    if pend is not None:
                        pt, phg, pet = pend
                        acc4 = emit_ctx(g, v_aug, pt, phg, pet)
                        emit_norm(g, ctxT_sb, pt, phg, acc4)
                    pend = (t, hg, et4)
                if g + 1 < NG:
                    ab[g + 1] = emit_a(g + 1, *xgn)
                pt, phg, pet = pend
                acc4 = emit_ctx(g, v_aug, pt, phg, pet)
                emit_norm(g, ctxT_sb, pt, phg, acc4)
                emit_c(g, ctxT_sb)

    return nc


_PROG_CACHE = {}


def _get_program(masked: bool):
    if masked not in _PROG_CACHE:
        _PROG_CACHE[masked] = _build_program(masked)
    return _PROG_CACHE[masked]


def _prep_host(values, w_qkv, b_qkv, w_lin, b_lin):
    """Host-side weight preprocessing (all cheap, shared across cores)."""
    scale = 1.0 / np.sqrt(DH)
    w_qkv = np.asarray(w_qkv, np.float32).copy()
    b_qkv = np.asarray(b_qkv, np.float32).copy()
    w_lin = np.asarray(w_lin, np.float32)
    b_lin = np.asarray(b_lin, np.float32)
    w_qkv[:DOUT] *= scale
    b_qkv[:DOUT] *= scale

    wqk = np.ascontiguousarray(
        w_qkv[:2 * DOUT].T.reshape(2, 128, 512)
    )  # [k-chunk, k-part, row]
    wv = np.ascontiguousarray(w_qkv[2 * DOUT:].T.reshape(2, 128, 256))
    wl = np.ascontiguousarray(w_lin.T.reshape(2, 128, 256))
    bqk = np.ascontiguousarray(b_qkv[:2 * DOUT].reshape(4, 128).T)
    b_v = b_qkv[2 * DOUT:]  # unscaled: only the q section was scaled above
    bfin = (w_lin @ b_v + b_lin).astype(np.float32)
    return wqk, wv, wl, bqk, bfin


def _run(values_padded, mask, w_arrs, trace=False):
    """values_padded: [N, 256] in track-padded order; mask: None or [N]."""
    wqk, wv, wl, bqk, bfin = w_arrs
    masked = mask is not None
    nc = _get_program(masked)

    in_maps = []
    for c in range(N_CORES):
        sl = slice(c * PC, (c + 1) * PC)
        xTc = np.ascontiguousarray(
            values_padded[sl].T.reshape(2, 128, PC)
        )
        m = dict(xT=xTc, wqk=wqk, wv=wv, wl=wl, bqk=bqk, bfin=bfin)
        if masked:
            mc_ = np.ascontiguousarray(mask[sl], np.float32)
            m["mkf"] = mc_
            m["mkp"] = np.ascontiguousarray(mc_.reshape(PC // 128, 128).T)
        in_maps.append(m)

    if trace:
        _ensure_ntff_hook()
    res = run_bass_kernel_spmd(nc, in_maps, list(range(N_CORES)), trace=trace)
    outp = np.concatenate([res.results[c]["out"] for c in range(N_CORES)], 0)
    return outp, res


LAST_RESULTS = None


def kernel(values, w_qkv, b_qkv, w_lin, b_lin, track_ids, n_tracks,
           num_heads, _trace=False):
    global LAST_RESULTS
    values = np.asarray(values, np.float32)
    track_ids = np.asarray(track_ids, np.int32)
    n_tracks_i = int(n_tracks)
    num_heads_i = int(num_heads)
    assert values.shape == (N, DIN) and n_tracks_i == T and num_heads_i == H, (
        "kernel compiled for N=65536, d=256, T=256, H=8"
    )

    w_arrs = _prep_host(values, w_qkv, b_qkv, w_lin, b_lin)

    counts = np.bincount(track_ids, minlength=T)
    equal = bool((counts == L).all())

    if equal:
        outp, res = _run(values, None, w_arrs, trace=_trace)
        LAST_RESULTS = res
        return outp

    # general sorted-ragged path: scatter to padded [T, L] grid on host,
    # run the same device kernel with padding masked out of K and V, then
    # gather back (mirroring jax's oob-drop scatter / clip gather).
    starts = np.concatenate([[0], np.cumsum(counts)[:-1]])
    pos = np.arange(N, dtype=np.int64) - starts[track_ids]
    keep = pos < L
    rows = track_ids.astype(np.int64) * L + np.minimum(pos, L - 1)
    padded = np.zeros((T * L, DIN), np.float32)
    padded[rows[keep]] = values[keep]
    mask = np.zeros(T * L, np.float32)
    mask[rows[keep]] = 1.0
    outp, res = _run(padded, mask, w_arrs, trace=_trace)
    LAST_RESULTS = res
    return np.ascontiguousarray(outp[rows])


# revision 22
# speedup vs baseline: 1.0316x; 1.0316x over previous
"""Trainium2 Bass kernel for a track-wise (ragged-sequence) attention layer.

Math (per track t of length L, per head h):
    qkv = values @ w_qkv.T + b_qkv                      # [N, 3*256]
    S   = q k^T / sqrt(Dh);  P = softmax(S);  ctx = P v
    out = ctx @ w_lin.T + b_lin

Device strategy (data parallel over tracks, 8 cores, no cross-core comm):
  - scores are computed directly transposed per head: ST = K @ Q^T, so
    exp(ST) is exactly the lhs^T the ctx matmul needs -- no PE transposes.
  - no max-subtraction in softmax (scores are ~N(0,1); exp is safe in fp32);
    the denominator is folded into the ctx matmul: lhsT = [V_h | ones], so
    one PE pass yields [ctx^T; rowsum]; 1/rowsum is computed as exp(-ln(x))
    on the scalar engine (ln+exp share one activation table set) and applied
    with four partition-offset DVE multiplies that also restack heads.
  - 1/sqrt(Dh) is folded into w_q/b_q on the host; b_v is folded into the
    final bias (softmax rows sum to 1): b_final = w_lin @ b_v + b_lin.
  - all matmuls run as float32r (full rate on the TRN2 PE; ~tf32 accuracy,
    measured end-to-end rel err ~3e-4).
  - emission is software-pipelined: the PE stream never waits on the same
    iteration's exp (ST matmuls of iteration i+1 precede ctx matmuls of
    iteration i), and the next group's projections fill the normalize tail.
"""

import os
import sys

import numpy as np

for _p in ("/opt/trn_rl_repo", "/root/.axon_site/_ro/trn_rl_repo"):
    if os.path.isdir(_p) and _p not in sys.path:
        sys.path.append(_p)

import concourse.bass as bass
import concourse.tile as tile
from concourse import mybir
from concourse.vector_clock import ScopedClock
from concourse.bass_utils import run_bass_kernel_spmd

F32 = mybir.dt.float32
F32R = mybir.dt.float32r
EXP = mybir.ActivationFunctionType.Exp
LOG = mybir.ActivationFunctionType.Ln
MUL = mybir.AluOpType.mult
ADD = mybir.AluOpType.add

N_CORES = 8
N, DIN, DOUT, H, T, L = 65536, 256, 256, 8, 256, 256
DH = DOUT // H          # 32
PC = N // N_CORES       # 8192 points per core
TPC = T // N_CORES      # 32 tracks per core
TPG = 4                 # tracks per group
NG = TPC // TPG         # 8 groups
GP = TPG * L            # 1024 points per group
MC_G = GP // 128        # 8 128-point chunks per group


class _TileContext(tile.TileContext):
    """TileContext whose final drain carries at most one semaphore wait per
    instruction (the walrus build in this container rejects multi-wait
    TPB_CTRL instructions)."""

    def _drain_and_barrier(self, tick_clock, wait_clock):
        super()._drain_and_barrier(tick_clock, wait_clock)
        self._split_multi_waits()

    def _split_multi_waits(self):
        nc = self.nc
        for f in nc.m.functions:
            for bb in f.blocks:
                changed = False
                new_insts = []
                for inst in bb.instructions:
                    si = inst.sync_info
                    if si is not None and len(si.on_wait) > 1:
                        waits = list(si.on_wait)
                        for w in waits[:-1]:
                            nop = mybir.InstNoOp(
                                name=f"I-{nc.next_id()}", ins=[], outs=[]
                            )
                            nop.engine = inst.engine
                            nop.sync_info = mybir.SyncInfo(
                                on_wait=[w], on_update=[]
                            )
                            new_insts.append(nop)
                        inst.sync_info = mybir.SyncInfo(
                            on_wait=[waits[-1]], on_update=list(si.on_update)
                        )
                        changed = True
                    new_insts.append(inst)
                if changed:
                    bb.instructions = new_insts


def _r(ap):
    return ap.bitcast(F32R)


def _ensure_ntff_hook():
    """The agent image's ``antenv`` lacks ``axon_hooks``; provide it so
    ``run_bass_kernel_spmd(trace=True)`` can register the axon NTFF hook."""
    try:
        import antenv.axon_hooks  # noqa: F401
        return
    except ImportError:
        pass
    import types

    import antenv

    mod = types.ModuleType("antenv.axon_hooks")
    _hook = [None]
    mod.set_axon_ntff_profile_hook = lambda h: _hook.__setitem__(0, h)
    mod.get_axon_ntff_profile_hook = lambda: _hook[0]
    sys.modules["antenv.axon_hooks"] = mod
    antenv.axon_hooks = mod
    try:
        from trn_agent_boot.trn_boot import _ntff_profile_via_ctypes

        mod.set_axon_ntff_profile_hook(
            _ntff_profile_via_ctypes("/opt/axon/libaxon_pjrt.so")
        )
    except Exception as e:  # pragma: no cover - tracing is best-effort
        print(f"ntff hook setup failed: {e}", file=sys.stderr)


def _build_program(masked: bool):
    nc = bass.Bass("TRN2", target_bir_lowering=False, debug=False,
                   num_devices=N_CORES)

    xT = nc.dram_tensor("xT", [2, 128, PC], F32R, kind="ExternalInput").ap()
    wqk = nc.dram_tensor("wqk", [2, 128, 512], F32R, kind="ExternalInput").ap()
    wv = nc.dram_tensor("wv", [2, 128, 256], F32R, kind="ExternalInput").ap()
    wl = nc.dram_tensor("wl", [2, 128, 256], F32R, kind="ExternalInput").ap()
    bqk = nc.dram_tensor("bqk", [128, 4], F32, kind="ExternalInput").ap()
    bfin = nc.dram_tensor("bfin", [256], F32, kind="ExternalInput").ap()
    if masked:
        mkf = nc.dram_tensor("mkf", [PC], F32, kind="ExternalInput").ap()
        mkp = nc.dram_tensor("mkp", [128, PC // 128], F32,
                             kind="ExternalInput").ap()
    out = nc.dram_tensor("out", [PC, DOUT], F32, kind="ExternalOutput").ap()

    with _TileContext(nc) as tc:
        with (
            tc.tile_pool(name="consts", bufs=1) as consts,
            tc.tile_pool(name="xg", bufs=2) as xg_pool,
            tc.tile_pool(name="qk", bufs=2) as qk_pool,
            tc.tile_pool(name="vsb", bufs=2) as v_pool,
            tc.tile_pool(name="et", bufs=3) as et_pool,
            tc.tile_pool(name="ctxT", bufs=2) as ctx_pool,
            tc.tile_pool(name="osb", bufs=2) as o_pool,
            tc.tile_pool(name="ps_st", bufs=2, space="PSUM") as ps_st,
            tc.tile_pool(name="ps_c4", bufs=2, space="PSUM") as ps_c4,
        ):
            # ---- constants -------------------------------------------------
            wqk_sb = consts.tile([128, 2, 512], F32R)
            nc.gpsimd.dma_start(out=wqk_sb, in_=wqk.rearrange("k p r -> p k r"))
            wv_sb = consts.tile([128, 2, 256], F32R)
            nc.gpsimd.dma_start(out=wv_sb, in_=wv.rearrange("k p r -> p k r"))
            wl_sb = consts.tile([128, 2, 256], F32R)
            nc.gpsimd.dma_start(out=wl_sb, in_=wl.rearrange("k p r -> p k r"))
            bqk_sb = consts.tile([128, 4], F32)
            nc.gpsimd.dma_start(out=bqk_sb, in_=bqk)
            bfin_sb = consts.tile([128, 256], F32)
            nc.gpsimd.dma_start(out=bfin_sb, in_=bfin.partition_broadcast(128))
            if masked:
                mkp_sb = consts.tile([128, PC // 128], F32)
                nc.gpsimd.dma_start(out=mkp_sb, in_=mkp)

            def emit_load(g):
                gsl = slice(g * GP, (g + 1) * GP)
                xg = xg_pool.tile([128, 2, GP], F32R, tag="xg", name=f"xg{g}")
                nc.gpsimd.dma_start(
                    out=xg, in_=xT[:, :, gsl].rearrange("k p n -> p k n")
                )
                mk_sb = None
                if masked:
                    mk_sb = et_pool.tile([128, GP], F32, tag="mk",
                                         name=f"mk{g}")
                    nc.gpsimd.dma_start(
                        out=mk_sb, in_=mkf[gsl].partition_broadcast(128)
                    )
                return xg, mk_sb

            def emit_a(g, xg, mk_sb):
                # A1: Q^T/K^T rows [512, GP]
                qk_sb = qk_pool.tile([128, 4, GP], F32R, tag="qk", name=f"qk{g}")
                for rr in range(4):
                    for n2 in range(GP // 512):
                        ps = ps_c4.tile([128, 512], F32, tag="c4",
                                        name=f"psa{g}_{rr}_{n2}")
                        for k in range(2):
                            nc.tensor.matmul(
                                ps,
                                wqk_sb[:, k, rr * 128:(rr + 1) * 128],
                                xg[:, k, n2 * 512:(n2 + 1) * 512],
                                start=(k == 0), stop=(k == 1),
                            )
                        nc.vector.tensor_scalar_add(
                            qk_sb[:, rr, n2 * 512:(n2 + 1) * 512],
                            ps, bqk_sb[:, rr:rr + 1],
                        )
                if masked:
                    for rr in (2, 3):  # zero K^T columns of padded slots
                        nc.vector.tensor_tensor(
                            qk_sb[:, rr, :], qk_sb[:, rr, :], mk_sb, MUL
                        )
                # A2: V -> v_aug [p, mc, head, [V(32) | ones(32)]]
                v_aug = v_pool.tile([128, MC_G, H, 64], F32R, tag="va", name=f"va{g}")
                nc.gpsimd.memset(v_aug[:, :, :, 32:64].bitcast(F32), 1.0)
                for mc in range(MC_G):
                    ps = ps_c4.tile([128, 512], F32, tag="c4",
                                    name=f"psv{g}_{mc}")
                    for k in range(2):
                        nc.tensor.matmul(
                            ps[:, 0:256],
                            xg[:, k, mc * 128:(mc + 1) * 128],
                            wv_sb[:, k, :],
                            start=(k == 0), stop=(k == 1),
                        )
                    if masked:
                        nc.vector.tensor_scalar_mul(
                            v_aug[:, mc, :, 0:32], ps[:, 0:256].rearrange(
                                "p (h d) -> p h d", h=H),
                            mkp_sb[:, g * MC_G + mc:g * MC_G + mc + 1],
                        )
                    else:
                        nc.vector.tensor_copy(
                            v_aug[:, mc, :, 0:32],
                            ps[:, 0:256].rearrange("p (h d) -> p h d", h=H),
                        )
                return qk_sb, v_aug

            def emit_st_exp(g, qk_sb, t, hg):
                # ST = K@Q^T then exp, two 2-head sub-batches
                tsl = slice(t * 256, (t + 1) * 256)
                et4 = et_pool.tile([128, 4, 512], F32R, tag="et",
                                   name=f"et{g}_{t}_{hg}")
                st2s = [ps_st.tile([128, 2, 512], F32, tag="st",
                                   name=f"st{g}_{t}_{hg}_{sb}")
                        for sb in range(2)]
                for j in range(2):
                    for hh in range(4):
                        po = hh * 32
                        nc.tensor.matmul(
                            st2s[hh // 2][:, hh % 2, j * 256:(j + 1) * 256],
                            qk_sb[po:po + 32, 2 + hg,
                                  t * 256 + j * 128:t * 256 + (j + 1) * 128],
                            qk_sb[po:po + 32, hg, tsl],
                            start=True, stop=True,
                            tile_position=(po, 0),
                        )
                for sb in range(2):
                    nc.scalar.activation(
                        et4[:, sb * 2:(sb + 1) * 2, :], st2s[sb], EXP
                    )
                return et4

            def emit_ctx(g, v_aug, t, hg, et4):
                acc4 = ps_c4.tile([64, 4, 256], F32, tag="c4",
                                  name=f"acc{g}_{t}_{hg}")
                for hh in range(4):
                    h = hg * 4 + hh
                    for j in range(2):
                        nc.tensor.matmul(
                            acc4[:, hh, :],
                            v_aug[:, t * 2 + j, h, :],
                            et4[:, hh, j * 256:(j + 1) * 256],
                            start=(j == 0), stop=(j == 1),
                        )
                return acc4

            def emit_norm(g, ctxT_sb, t, hg, acc4):
                tsl = slice(t * 256, (t + 1) * 256)
                ln4 = et_pool.tile([32, 4, 256], F32, tag="ln",
                                   name=f"ln{g}_{t}_{hg}")
                nc.scalar.activation(ln4, acc4[32:64, :, :], LOG)
                rcp4 = et_pool.tile([32, 4, 256], F32, tag="rcp",
                                    name=f"rcp{g}_{t}_{hg}")
                nc.scalar.activation(rcp4, ln4, EXP, scale=-1.0)
                for hh in range(4):
                    nc.vector.tensor_tensor(
                        ctxT_sb[hh * 32:(hh + 1) * 32, hg, tsl],
                        acc4[0:32, hh, :], rcp4[:, hh, :], MUL
                    )

            def emit_c(g, ctxT_sb):
                gsl = slice(g * GP, (g + 1) * GP)
                o_sb = o_pool.tile([128, MC_G, 256], F32, tag="o", name=f"o{g}")
                for mc in range(MC_G):
                    ps = ps_c4.tile([128, 512], F32, tag="c4",
                                    name=f"psc{g}_{mc}")
                    for kc in range(2):
                        nc.tensor.matmul(
                            ps[:, 0:256],
                            ctxT_sb[:, kc, mc * 128:(mc + 1) * 128],
                            wl_sb[:, kc, :],
                            start=(kc == 0), stop=(kc == 1),
                        )
                    nc.vector.tensor_tensor(
                        o_sb[:, mc, :], ps[:, 0:256], bfin_sb, ADD
                    )
                nc.gpsimd.dma_start(
                    out=out[gsl, :].rearrange("(m p) n -> p m n", p=128),
                    in_=o_sb,
                )

            # software-pipelined schedule: the PE stream never waits on the
            # same iteration's exp -- ST MMs of iter i+1 are emitted before
            # the ctx MMs of iter i; the next group's A-phase is hoisted
            # before the current group's C-phase.
            xg0 = emit_load(0)
            qk, va = emit_a(0, *xg0)
            ab = {0: (qk, va)}
            iters = [(t, hg) for t in range(TPG) for hg in (0, 1)]
            for g in range(NG):
                qk_sb, v_aug = ab.pop(g)
                ctxT_sb = ctx_pool.tile([128, 2, GP], F32R, tag="ctxT", name=f"ctxT{g}")
                if g + 1 < NG:
                    xgn = emit_load(g + 1)
                pend = None
                for i, (t, hg) in enumerate(iters):
                    et4 = emit_st_exp(g, qk_sb, t, hg)
                    if pend is not None:
                        pt, phg, pet = pend
                        acc4 = emit_ctx(g, v_aug, pt, phg, pet)
                        emit_norm(g, ctxT_sb, pt, phg, acc4)
                    pend = (t, hg, et4)
                if g + 1 < NG:
                    ab[g + 1] = emit_a(g + 1, *xgn)
                pt, phg, pet = pend
                acc4 = emit_ctx(g, v_aug, pt, phg, pet)
                emit_norm(g, ctxT_sb, pt, phg, acc4)
                emit_c(g, ctxT_sb)

    return nc


_PROG_CACHE = {}


def _get_program(masked: bool):
    if masked not in _PROG_CACHE:
        _PROG_CACHE[masked] = _build_program(masked)
    return _PROG_CACHE[masked]


def _prep_host(values, w_qkv, b_qkv, w_lin, b_lin):
    """Host-side weight preprocessing (all cheap, shared across cores)."""
    scale = 1.0 / np.sqrt(DH)
    w_qkv = np.asarray(w_qkv, np.float32).copy()
    b_qkv = np.asarray(b_qkv, np.float32).copy()
    w_lin = np.asarray(w_lin, np.float32)
    b_lin = np.asarray(b_lin, np.float32)
    w_qkv[:DOUT] *= scale
    b_qkv[:DOUT] *= scale

    wqk = np.ascontiguousarray(
        w_qkv[:2 * DOUT].T.reshape(2, 128, 512)
    )  # [k-chunk, k-part, row]
    wv = np.ascontiguousarray(w_qkv[2 * DOUT:].T.reshape(2, 128, 256))
    wl = np.ascontiguousarray(w_lin.T.reshape(2, 128, 256))
    bqk = np.ascontiguousarray(b_qkv[:2 * DOUT].reshape(4, 128).T)
    b_v = b_qkv[2 * DOUT:]  # unscaled: only the q section was scaled above
    bfin = (w_lin @ b_v + b_lin).astype(np.float32)
    return wqk, wv, wl, bqk, bfin


def _run(values_padded, mask, w_arrs, trace=False):
    """values_padded: [N, 256] in track-padded order; mask: None or [N]."""
    wqk, wv, wl, bqk, bfin = w_arrs
    masked = mask is not None
    nc = _get_program(masked)

    in_maps = []
    for c in range(N_CORES):
        sl = slice(c * PC, (c + 1) * PC)
        xTc = np.ascontiguousarray(
            values_padded[sl].T.reshape(2, 128, PC)
        )
        m = dict(xT=xTc, wqk=wqk, wv=wv, wl=wl, bqk=bqk, bfin=bfin)
        if masked:
            mc_ = np.ascontiguousarray(mask[sl], np.float32)
            m["mkf"] = mc_
            m["mkp"] = np.ascontiguousarray(mc_.reshape(PC // 128, 128).T)
        in_maps.append(m)

    if trace:
        _ensure_ntff_hook()
    res = run_bass_kernel_spmd(nc, in_maps, list(range(N_CORES)), trace=trace)
    outp = np.concatenate([res.results[c]["out"] for c in range(N_CORES)], 0)
    return outp, res


LAST_RESULTS = None


def kernel(values, w_qkv, b_qkv, w_lin, b_lin, track_ids, n_tracks,
           num_heads, _trace=False):
    global LAST_RESULTS
    values = np.asarray(values, np.float32)
    track_ids = np.asarray(track_ids, np.int32)
    n_tracks_i = int(n_tracks)
    num_heads_i = int(num_heads)
    assert values.shape == (N, DIN) and n_tracks_i == T and num_heads_i == H, (
        "kernel compiled for N=65536, d=256, T=256, H=8"
    )

    w_arrs = _prep_host(values, w_qkv, b_qkv, w_lin, b_lin)

    counts = np.bincount(track_ids, minlength=T)
    equal = bool((counts == L).all())

    if equal:
        outp, res = _run(values, None, w_arrs, trace=_trace)
        LAST_RESULTS = res
        return outp

    # general sorted-ragged path: scatter to padded [T, L] grid on host,
    # run the same device kernel with padding masked out of K and V, then
    # gather back (mirroring jax's oob-drop scatter / clip gather).
    starts = np.concatenate([[0], np.cumsum(counts)[:-1]])
    pos = np.arange(N, dtype=np.int64) - starts[track_ids]
    keep = pos < L
    rows = track_ids.astype(np.int64) * L + np.minimum(pos, L - 1)
    padded = np.zeros((T * L, DIN), np.float32)
    padded[rows[keep]] = values[keep]
    mask = np.zeros(T * L, np.float32)
    mask[rows[keep]] = 1.0
    outp, res = _run(padded, mask, w_arrs, trace=_trace)
    LAST_RESULTS = res
    return np.ascontiguousarray(outp[rows])
